# revision 24
# baseline (speedup 1.0000x reference)
"""Cross-attention Trainium2 kernel (8 NeuronCores, data-parallel).

Problem: B=4, C=64, H=64, W=64.
  q = conv1x1(v1, wq, bq); k = conv1x1(v2, wk, bk); v = conv1x1(v2, wv, bv)
  tokens n = (c, h) pairs (N = C*H = 4096), feature dim = W = 64
  out = softmax(q @ k^T) @ v

Sharding: core i handles batch b = i//2 and the q-token half h in
[32*(i%2), 32*(i%2+1)).  Every core needs the full v2[b] (k/v side) but only
its h-slice of v1[b] (q side).  No collectives.

Design (v3; v2 trace showed the loop paced by exp and DMA arriving at 16us):
  - inputs host-prepped fp16, minimal bytes (~0.9MB/core): x1p/x2p
    h-parity-packed [hp*64+c, (hh w)], one small "blob" carrying the three
    pre-transposed weight matrices (doubled on both partition halves) and
    the q/k bias patterns, a fp32 identity, a per-partition bv column.
    V is projected straight from x2p (no channel-major copy of x2).
  - warm-up burst (fp32 matmuls off a vector-memset tile) keeps the PE
    busy from boot so the HAM clock gate opens before the real compute;
    filler matmuls bridge the projection->pass-0 dependency gap so the
    PE never idles >3.4us (which would re-throttle it to 1.2GHz).
  - Q/K projections stream fp16 at full PE rate (FWL); q/k biases (which
    ride the token axis with period 64) are added by GpSimd tensor_tensor
    per 512-col chunk, pipelined with the projection copies.  V bias
    rides the output channel -> folded into the PSUM->SBUF copies
    (ScalarE activation bias / VectorE tensor_scalar_add, per-partition).
  - main loop: 64 global slots (4 passes x 16 j-pairs), scores lookahead
    2 with THREE score buffers (with two, scores(s+2) stalled ~800ns on
    exp(s) freeing its buffer - the measured v2 slot was 873ns vs the
    ~700ns exp-throughput floor).  exp alternates ScalarE LUT / VectorE
    Schraudolph (the two engines together are the ~11us/pass floor; the
    GpSimd has no PSUM port so it cannot help).  The two PV matmuls of a
    slot accumulate into TWO PSUM banks (even jb -> A, odd jb -> B) so
    they stream back-to-back with no same-bank RAW stall.
  - drain of pass d during pass d+1: copy A (scalar) || copy B (vector)
    at p==0, 8 accumulating PE transposes (A-chunk + B-chunk summed in
    PSUM - the A+B add costs zero DVE time) into a borrowed score tile
    at p==4, reciprocals (vector) at p==5, per-partition scaled copies
    (2 scalar / 2 vector) into a staging tile at p==6.  ONE output DMA
    at the very end with (r w) 1KB-contiguous runs (v2's per-pass DMAs
    of 256B elements left a 6.5us exposed tail).
"""

import numpy as np

B, C, H, W = 4, 64, 64, 64
HH = H // 2            # h-rows per core (q-token half)
NQ = C * HH            # q tokens per core = 2048
NK = C * H             # k tokens = 4096
JB = NK // 128         # 32 j-blocks of 128 k-tokens
NP = JB // 2           # 16 row-packed j-block pairs
IP = 512               # i-span per pass
NPASS = NQ // IP       # 4
NSLOT = NPASS * NP     # 64 global pipeline slots
NCORES = 8
LOOKAHEAD = 3

LOG2E = 1.4426950408889634
SCH_SCALE = 128.0 * LOG2E
SCH_BIAS = 16256.0 - 7.0   # centered so the sawtooth ratio has mean ~1
N_WARM = 20                # HAM warm-up matmuls (fp32, ~430ns each cold)
N_FILL = 14                # PE filler matmuls bridging proj -> pass 0

_CACHE = {}


def _build_nc():
    from contextlib import ExitStack

    import concourse.bass as bass
    import concourse.tile as tile
    from concourse import bacc, mybir
    from concourse.bass import ts

    F32 = mybir.dt.float32
    F16 = mybir.dt.float16
    BF16 = mybir.dt.bfloat16
    I16 = mybir.dt.int16
    AF = mybir.ActivationFunctionType
    ALU = mybir.AluOpType

    nc = bacc.Bacc(trn_type="TRN2", target_bir_lowering=False)

    # blob cols: wqT2 0:64 | wkT2 64:128 | wvT2 128:192 | brdk pattern
    # 192:256 (bk tiled 1x, widened on-chip) | brdq pattern 256:768
    # (repeat(bq, 8) matching the q-token order i = o*8 + uu)
    x1p_d = nc.declare_dram_parameter("x1p", [128, (HH // 2) * W], F16, False)
    x2p_d = nc.declare_dram_parameter("x2p", [128, (H // 2) * W], F16, False)
    blob_d = nc.declare_dram_parameter("blob", [128, 768], F16, False)
    bvp_d = nc.declare_dram_parameter("bvp", [128, 1], F32, False)
    ident_d = nc.declare_dram_parameter("ident", [128, 128], F32, False)
    out_d = nc.declare_dram_parameter("out", [C, HH, W], F16, True)

    with ExitStack() as ctx:
        tc = ctx.enter_context(tile.TileContext(nc))
        cp = ctx.enter_context(tc.tile_pool(name="const", bufs=1))

        # prewarm the exp table set while input DMAs run
        warm = cp.tile([128, 2], F32)
        nc.vector.memset(warm[:, :], 0.0)
        nc.scalar.activation(warm[:, 0:1], warm[:, 1:2], AF.Exp)

        # warm-up feed tile: no gpsimd dependency, so the HAM burst starts
        # as soon as the engines boot
        wfill = cp.tile([128, 128], F32)
        nc.vector.memset(wfill[:, :], 0.0)

        x1p = cp.tile([128, (HH // 2) * W], F16)
        x2p = cp.tile([128, (H // 2) * W], F16)
        blob = cp.tile([128, 768], F16)
        bvp = cp.tile([128, 1], F32)
        ident = cp.tile([128, 128], F32)
        brdk = cp.tile([128, 512], F16)

        qT2 = cp.tile([128, NQ], F16)        # (w, i) doubled on both halves
        kT2 = cp.tile([128, NK // 2], F16)   # (w, j) even jb lower / odd upper
        vf = cp.tile([128, JB, 65], BF16)    # v-tokens on partitions, col64=1
        # per-pass drained accumulators (A in cols 0:512, B in 512:1024)
        outT0 = cp.tile([C + 1, 2 * IP], F32)
        outT1 = cp.tile([C + 1, 2 * IP], F32)
        outT2 = cp.tile([C + 1, 2 * IP], F32)
        outT3 = cp.tile([C + 1, 2 * IP], F32)
        obig = cp.tile([128, 4, NPASS, W], F16)  # normalized output staging

        wqt2 = blob[:, 0:64]
        wkt2 = blob[:, 64:128]
        wtv2 = blob[:, 128:192]
        brdq = blob[:, 256:768]

        # DMA issue split across engine queues; most-critical first.
        nc.sync.dma_start(blob[:, :], blob_d[:, :])
        nc.sync.dma_start(x1p[:, :], x1p_d[:, :])
        nc.sync.dma_start(ident[:, :], ident_d[:, :])
        nc.gpsimd.dma_start(x2p[:, :], x2p_d[:, :])
        nc.gpsimd.dma_start(bvp[:, :], bvp_d[:, :])
        nc.gpsimd.memset(vf[:, :, 64:65], 1.0)

        # widen the k bias pattern (period 64) to a 512-col stt operand
        nc.vector.tensor_copy(brdk[:, 0:64], blob[:, 192:256])
        nc.vector.tensor_copy(brdk[:, 64:128], brdk[:, 0:64])
        nc.vector.tensor_copy(brdk[:, 128:256], brdk[:, 0:128])
        nc.vector.tensor_copy(brdk[:, 256:512], brdk[:, 0:256])

        # ---- HAM warm-up burst (PE busy from boot; ~6us of fp32 matmuls)
        with tc.tile_pool(name="ppw", bufs=1, space="PSUM") as ppw:
            wps = ppw.tile([128, 128], F32, tag="warm")
            for _ in range(N_WARM):
                nc.tensor.matmul(wps[:, :], lhsT=wfill[:, :], rhs=wfill[:, :],
                                 start=True, stop=True)

        # ---- projections: Q first (critical), then K groups, then V ----
        with (
            tc.tile_pool(name="ppq", bufs=1, space="PSUM") as ppq,
            tc.tile_pool(name="ppkv", bufs=3, space="PSUM") as ppkv,
        ):
            def k_mms(g):
                ps = ppkv.tile([128, 1024], F32, tag="kv", name="kps")
                for uu in range(4):
                    for hp in range(2):
                        nc.tensor.matmul(
                            ps[:, hp * 512 + uu * C:][:, 0:C],
                            lhsT=x2p[ts(hp, C), 512 * g + 128 * uu:][:, 0:128],
                            rhs=wkt2[ts(hp, C), :],
                            start=True, stop=True,
                        )
                return ps

            def v_mms(hp, c2):
                # out[o, (hh w)] for h = 2*hh + hp, hh in [16*c2, 16*c2+16)
                ps = ppkv.tile([64, 1024], F32, tag="kv", name="vps")
                for j in range(2):
                    nc.tensor.matmul(
                        ps[:, ts(j, 512)],
                        lhsT=wtv2[ts(hp, C), :],
                        rhs=x2p[ts(hp, C), 1024 * c2 + 512 * j:][:, 0:512],
                        start=True, stop=True,
                    )
                return ps

            def k_copies(g, ps, eng_stt):
                for hh2 in range(2):
                    src = ps[ts(hh2, C), :].rearrange(
                        "p (hp uu o) -> p uu hp o", hp=2, o=C
                    )
                    dst = kT2[64 * hh2: 64 * hh2 + C, ts(g, 512)].rearrange(
                        "p (uu hp o) -> p uu hp o", uu=4, hp=2
                    )
                    if hh2 == 0:
                        nc.vector.tensor_copy(dst, src[:, 0:4, :, :])
                    else:
                        nc.scalar.copy(dst, src[:, 0:4, :, :])
                eng_stt.tensor_tensor(kT2[:, ts(g, 512)], kT2[:, ts(g, 512)],
                                      brdk[:, :], ALU.add)

            def v_copies(hp, c2, ps):
                # vf[64*hp + o, hh, w] = v[o, 2*hh+hp, w] + bv[o]
                src = ps[:, :].rearrange("p (hh w) -> p hh w", w=W)
                dst = vf[ts(hp, C), 16 * c2: 16 * c2 + 16, 0:W]
                if hp == 0:
                    nc.scalar.activation(dst, src, AF.Identity,
                                         bias=bvp[0:C, 0:1])
                else:
                    nc.vector.tensor_scalar_add(dst, src, bvp[0:C, 0:1])

            def q_copy(hh2):
                # token order within a 1024-chunk: (hp, o, uu) so the
                # drained output tiles DMA out as contiguous 4KB runs
                src = psq[ts(hh2, C), :].rearrange(
                    "p (hp uu o) -> p hp uu o", hp=2, o=C
                )
                dst = qT2[0:C, ts(hh2, 1024)].rearrange(
                    "p (hp o uu) -> p hp uu o", hp=2, uu=8
                )
                nc.scalar.copy(dst, src)

            def q_bias(ihc, eng_stt):
                eng_stt.tensor_tensor(qT2[0:C, ts(ihc, 512)],
                                      qT2[0:C, ts(ihc, 512)],
                                      brdq[0:C, :], ALU.add)
                eng_stt.tensor_copy(qT2[C:2 * C, ts(ihc, 512)],
                                    qT2[0:C, ts(ihc, 512)])

            # Q: token order i = hh2*1024 + hp*512 + uu*64 + o, h=4uu+2hh2+hp
            psq = ppq.tile([128, 1024], F32, tag="q")
            for uu in range(8):
                for hp in range(2):
                    nc.tensor.matmul(
                        psq[:, hp * 512 + uu * C:][:, 0:C],
                        lhsT=x1p[ts(hp, C), ts(uu, 128)],
                        rhs=wqt2[ts(hp, C), :],
                        start=True, stop=True,
                    )
            # K: jb = 8g + 2uu + hh2, scores pair p = 4g + uu.
            # Early biases (gating pass-0 slots) on the vector engine;
            # late ones (pass>=1 / slot>=8) on the slow-but-idle gpsimd.
            kp0 = k_mms(0)
            kp1 = k_mms(1)
            q_copy(0)
            k_copies(0, kp0, nc.vector)
            q_bias(0, nc.vector)
            kp2 = k_mms(2)
            q_bias(1, nc.gpsimd)
            q_copy(1)
            k_copies(1, kp1, nc.vector)
            kp3 = k_mms(3)
            k_copies(2, kp2, nc.gpsimd)
            vp00 = v_mms(0, 0)
            k_copies(3, kp3, nc.gpsimd)
            q_bias(2, nc.gpsimd)
            q_bias(3, nc.gpsimd)
            vp10 = v_mms(1, 0)
            v_copies(0, 0, vp00)
            vp01 = v_mms(0, 1)
            v_copies(1, 0, vp10)
            vp11 = v_mms(1, 1)
            v_copies(0, 1, vp01)
            v_copies(1, 1, vp11)

        # ---- main attention loop: 64 global slots, psA/psB bank split ----
        with (
            tc.tile_pool(name="accp", bufs=1, space="PSUM") as accp,
            tc.tile_pool(name="sp", bufs=3, space="PSUM") as sp,
            tc.tile_pool(name="ppool", bufs=4) as p_pool,
            tc.tile_pool(name="rpool", bufs=2) as r_pool,
        ):
            accA = accp.tile([128, IP], F32, tag="accA", name="accA")
            accB = accp.tile([128, IP], F32, tag="accB", name="accB")
            sps_ring = {}
            pt_ring = {}
            tps_ring = {}
            r_ring = {}

            # filler matmuls: keep the PE warm while the projection
            # copies/biases run on the DVE engines
            for f in range(N_FILL):
                dst = accA if f % 2 == 0 else accB
                nc.tensor.matmul(dst[:, 0:128], lhsT=wfill[:, :],
                                 rhs=wfill[:, :], start=True, stop=True)

            def emit_scores(s):
                ih, p = divmod(s, NP)
                i0 = ih * IP
                sps = sp.tile([128, 2 * IP], F32, tag="scores", name="sps")
                for blk in range(2):
                    half = 64 * blk
                    nc.tensor.matmul(
                        sps[:, ts(blk, IP)],
                        lhsT=kT2[half: half + 64, ts(p, 128)],
                        rhs=qT2[half: half + 64, i0: i0 + IP],
                        start=True, stop=True,
                    )
                sps_ring[s] = sps

            def emit_exp(s):
                sps = sps_ring.pop(s)
                pt = p_pool.tile([128, 2 * IP], BF16, tag="p", name="pt")
                if s % 2 == 0:
                    nc.scalar.activation(pt[:, :], sps[:, :], AF.Exp)
                else:
                    # Schraudolph bit-trick exp on the DVE
                    nc.vector.tensor_scalar(
                        pt[:, :].bitcast(I16), sps[:, :], SCH_SCALE, SCH_BIAS,
                        ALU.mult, ALU.add,
                    )
                pt_ring[s] = pt

            def emit_pv(s):
                ih, p = divmod(s, NP)
                pt = pt_ring.pop(s)
                for blk, dst in ((0, accA), (1, accB)):
                    nc.tensor.matmul(
                        dst[0:C + 1, :],
                        lhsT=vf[:, 2 * p + blk, :],
                        rhs=pt[:, ts(blk, IP)],
                        start=(p == 0), stop=(p == NP - 1),
                    )

            outT = [outT0, outT1, outT2, outT3]

            # in-loop drain of pass d: ONLY the two PSUM->SBUF copies (the
            # transposes/recips/muls run post-loop so they never borrow a
            # scores-ring buffer, whose WAR semaphores would otherwise gate
            # the ring on slow drain consumers)
            def emit_copy(d):
                nc.scalar.copy(outT[d][:, 0:IP], accA[0:C + 1, :])
                nc.vector.tensor_copy(outT[d][:, IP:2 * IP], accB[0:C + 1, :])

            def emit_transposes(d):
                # accumulating transposes: ps[.,t] = (A-chunk + B-chunk)^T
                tps = sp.tile([128, 2 * IP], F32, tag="scores", name="tps")
                tps_ring[d] = tps
                for t in range(4):
                    for c0, st in ((0, True), (IP, False)):
                        nc.tensor.matmul(
                            tps[:, 65 * t: 65 * t + 65],
                            lhsT=outT[d][:, c0 + 128 * t: c0 + 128 * t + 128],
                            rhs=ident[0:C + 1, 0:C + 1],
                            is_transpose=True, start=st, stop=not st,
                        )

            def emit_recips(d):
                tps = tps_ring[d]
                rec = r_pool.tile([128, 4], F32, tag="rec", name="rec")
                r_ring[d] = rec
                for t in range(4):
                    nc.vector.reciprocal(rec[:, t: t + 1],
                                         tps[:, 65 * t + 64: 65 * t + 65])

            def emit_muls(d):
                tps = tps_ring.pop(d)
                rec = r_ring.pop(d)
                for t in range(4):
                    src = tps[:, 65 * t: 65 * t + C]
                    if t < 2:
                        nc.scalar.activation(obig[:, t, d, :], src, AF.Copy,
                                             scale=rec[:, t: t + 1])
                    else:
                        nc.vector.tensor_scalar_mul(obig[:, t, d, :], src,
                                                    rec[:, t: t + 1])

            for s0 in range(LOOKAHEAD):
                emit_scores(s0)
            for s in range(NSLOT):
                ih, p = divmod(s, NP)
                if s + LOOKAHEAD < NSLOT:
                    emit_scores(s + LOOKAHEAD)
                emit_exp(s)
                if ih > 0 and p == 0:
                    emit_copy(ih - 1)
                emit_pv(s)

            emit_copy(NPASS - 1)
            for d in range(NPASS):
                emit_transposes(d)
                emit_recips(d)
                emit_muls(d)

            # output DMAs: with the (o, uu) q-token order, drain tile t of
            # pass r holds out[16t + p//8, 4*(p%8) + r, :] -> per tile t the
            # DRAM block out[16t:16t+16, :, :] is written as 16 contiguous
            # 4KB runs.  One DMA per t, striped across engine queues.
            dest4 = out_d[:, :, :].rearrange(
                "o (uu r) w -> (o uu) r w", r=NPASS
            )
            for t, eng in ((0, nc.sync), (1, nc.gpsimd), (2, nc.scalar),
                           (3, nc.sync)):
                eng.dma_start(dest4[128 * t: 128 * t + 128, :, :],
                              obig[:, t, :, :])

    nc.compile()
    return nc


def _get_nc():
    if "nc" not in _CACHE:
        _CACHE["nc"] = _build_nc()
    return _CACHE["nc"]


def _in_maps(v1, v2, wq, bq, wk, bk, wv, bv):
    f32, f16 = np.float32, np.float16
    wq = np.asarray(wq, f32)
    wk = np.asarray(wk, f32)
    wv = np.asarray(wv, f32)
    bq = np.asarray(bq, f32)
    bk = np.asarray(bk, f32)
    bv = np.asarray(bv, f32)
    blob = np.concatenate([
        np.tile(wq.T, (2, 1)),
        np.tile(wk.T, (2, 1)),
        np.tile(wv.T, (2, 1)),
        np.tile(bk.reshape(1, C), (128, 1)),
        np.tile(np.repeat(bq, 8).reshape(1, 512), (128, 1)),
    ], axis=1).astype(f16)
    blob = np.ascontiguousarray(blob)
    bvp = np.ascontiguousarray(np.tile(bv, 2).reshape(128, 1).astype(f32))
    ident = np.eye(128, dtype=f32)
    maps = []
    for core in range(NCORES):
        b, half = divmod(core, 2)
        x1s = np.asarray(v1[b, :, half * HH: (half + 1) * HH, :], f32)
        x2s = np.asarray(v2[b], f32)
        x1p = np.ascontiguousarray(
            x1s.reshape(C, HH // 2, 2, W).transpose(2, 0, 1, 3)
            .reshape(128, (HH // 2) * W).astype(f16)
        )
        x2p = np.ascontiguousarray(
            x2s.reshape(C, H // 2, 2, W).transpose(2, 0, 1, 3)
            .reshape(128, (H // 2) * W).astype(f16)
        )
        maps.append({
            "x1p": x1p, "x2p": x2p, "blob": blob, "bvp": bvp, "ident": ident,
        })
    return maps


def _gather(results, v1):
    out = np.zeros((B, C, H, W), dtype=np.float32)
    for core in range(NCORES):
        b, half = divmod(core, 2)
        out[b, :, half * HH: (half + 1) * HH, :] = (
            results[core]["out"].astype(np.float32)
        )
    return out


def _run(trace=False, **inputs):
    from concourse.bass_utils import run_bass_kernel_spmd

    nc = _get_nc()
    maps = _in_maps(**inputs)
    res = run_bass_kernel_spmd(
        nc, maps, core_ids=list(range(NCORES)), trace=trace
    )
    return _gather(res.results, inputs["v1"]), res


def kernel(**inputs):
    out, _ = _run(trace=False, **inputs)
    return out
